# revision 1
# baseline (speedup 1.0000x reference)
"""CRF loss via separable factorization on 8 Trainium2 NeuronCores.

Math: K[i,j] = Kspat[i,j] * sF_i sF_j exp(w_i.w_j), w = I/BETA,
sF = exp(-|w|^2/2), Kspat = Gx (x) Gy (x) Gz (exact separable Gaussian).
exp(w_i.w_j) ~= sum_a Phi_a(w_i) Phi_a(w_j) (degree-1 Taylor, P=4 channels;
measured rel err of the final loss ~3e-4 on these inputs vs 2e-2 budget).

  gauss_filter(v)_i = sum_a Phis_a,i * [Kspat @ (Phis_a * v)]_i,  Phis = Phi*sF

so each filter application (norm pass + message pass) is a P-channel separable
spatial filter: O(N*P) instead of O(N^2).

Sharding: core k -> (batch k//4, softmax-channel k%4). Pass 1 (norm) is
replicated within each 4-core batch group (no collectives); pass 2 handles the
core's own channel. The device ships [18, 648] = (FP | n) in bf16; the host
computes loss_part = sum(FP * n * (1-H1c)) and sums the 8 partials.

Per-core device pipeline (one stack of 4 volumes on partitions):
  A-layout [128, 384]: row 18v+x (v<4), col 18y+z (<324, padded to 384)
  x-filter+transpose in ONE matmul per chunk (data stationary, block-diag Gx
  streaming):  XP[m] = PhisA_chunk_m.T @ bdGx   -> B-layout (yz on partitions)
  yz-filter+transpose-back (B chunk stationary, kron(Gy,Gz) streaming), split
  into column halves with separate PSUM accumulators so downstream work
  chases half 0:
               A[h] += B[m].T @ Gyz[m][:, half_h]  -> A-layout again
  r = sel.T @ (PhisA * A1); n = (r+eps)^-1/2 (ScalarE abs-rsqrt table)
  NREP = selrep.T @ n   (bf16 matmul broadcasts n across the 4 vol slots)
  W2 = PhisH * NREP (PhisH = Phis*H1c folded on host) -> filters -> A2
  FP = sel.T @ (PhisA * A2)
All post-filter chains are pipelined in column halves.
"""

import math

import numpy as np
import ml_dtypes

import concourse.bass as bass
import concourse.bacc as bacc
import concourse.tile as tile
import concourse.mybir as mybir
import concourse.bass_utils as bass_utils
from concourse.hw_specs import get_activation_tables

ALPHA = 5.0
BETA = 5.0
EPS = 1e-20

B = 2
C = 4
XD = 18
N = XD ** 3
YZP = 384  # padded flat yz (3*128)
PAW = 480  # phisAM width: 0:384 PhisA, 384:456 bdGx, 456:474 sel

ALPHAS = [(0, 0, 0), (1, 0, 0), (0, 1, 0), (0, 0, 1)]
P = len(ALPHAS)
NS18 = 18 * P  # 72
HL = 128  # column half (chunk-aligned)

F32 = mybir.dt.float32
BF16 = mybir.dt.bfloat16
BF = ml_dtypes.bfloat16

TRACE = False
LAST_RESULT = None
USE_ARSQRT = True

_compiled = {}


def _build():
    nc = bacc.Bacc("TRN2", target_bir_lowering=False, debug=False, num_devices=8)

    phisAM = nc.dram_tensor("phisAM", [128, PAW], BF16, kind="ExternalInput")
    h1cb = nc.dram_tensor("h1cb", [18, 324], BF16, kind="ExternalInput")
    gyzA = nc.dram_tensor("gyzA", [128, 648], BF16, kind="ExternalInput")
    gyzB = nc.dram_tensor("gyzB", [68, 324], BF16, kind="ExternalInput")
    selrepb = nc.dram_tensor("selrepb", [18, 128], BF16, kind="ExternalInput")
    outn = nc.dram_tensor("outn", [18, 324], BF16, kind="ExternalOutput")
    outf = nc.dram_tensor("outf", [18, 324], BF16, kind="ExternalOutput")

    with tile.TileContext(nc) as tc:
        with (
            tc.tile_pool(name="const", bufs=1) as cp,
            tc.tile_pool(name="xp", bufs=3, space="PSUM") as xpp,
            tc.tile_pool(name="ap", bufs=2, space="PSUM") as app,
            tc.tile_pool(name="mp", bufs=1, space="PSUM") as mp,
        ):
            phisAM_sb = cp.tile([128, PAW], BF16)
            h1cb_sb = cp.tile([18, 324], BF16)
            nh_sb = cp.tile([18, 324], BF16)
            gyzA_sb = cp.tile([128, 648], BF16)
            gyzB_sb = cp.tile([68, 324], BF16)
            selrepb_sb = cp.tile([18, 128], BF16)
            eps_sb = cp.tile([18, 1], F32)
            B_sb = cp.tile([128, 384], BF16)
            W2_sb = cp.tile([128, YZP], BF16)
            FS_sb = cp.tile([72, 324], BF16)
            outb_sb = cp.tile([18, 648], BF16)  # cols 0:324 FP, 324:648 n
            lnr_sb = cp.tile([18, 324], F32)
            n_view = outb_sb[:, 324:648]
            bdgx_v = phisAM_sb[:, 384:384 + NS18]
            sel_v = phisAM_sb[0:NS18, 456:474]

            # Preload the ACT table set so no switches land mid-kernel.
            _tabs = list(get_activation_tables("gen3"))
            _nlx = _tabs.index(
                "abs_reciprocal_sqrt_and_small" if USE_ARSQRT
                else "natural_log_exp_and_others"
            )
            nc.scalar.add_instruction(
                mybir.InstLoadActFuncSet(
                    name=f"I-{nc.next_id()}", act_func_set_id=_nlx
                )
            )

            # ---- input DMAs: one hw queue per issuing engine; arrivals
            # sequenced/balanced to match consumption order ----
            nc.sync.dma_start(phisAM_sb[0:64, :], phisAM[0:64, :])
            nc.scalar.dma_start(phisAM_sb[64:128, :], phisAM[64:128, :])
            nc.gpsimd.dma_start(gyzB_sb[:], gyzB[:])
            nc.sync.dma_start(gyzA_sb[0:48, :], gyzA[0:48, :])
            nc.scalar.dma_start(gyzA_sb[48:96, :], gyzA[48:96, :])
            nc.gpsimd.dma_start(gyzA_sb[96:128, :], gyzA[96:128, :])
            nc.gpsimd.dma_start(selrepb_sb[:], selrepb[:])
            nc.gpsimd.dma_start(h1cb_sb[:], h1cb[:])

            # ---- init ----
            nc.vector.memset(W2_sb[:], 0.0)
            nc.vector.memset(B_sb[:], 0.0)
            nc.vector.memset(eps_sb[:], EPS)

            def filters(src_sb, tag):
                """src_sb [128, >=384] (A-layout) -> two psum halves
                [128, 162] (cols 0:162 and 162:324 of the filtered A)."""
                for m in range(3):
                    XPm = xpp.tile([128, 128], F32, tag="xp", name=f"XP{tag}{m}")
                    nc.tensor.matmul(
                        XPm[:, 0:NS18],
                        src_sb[:, 128 * m:128 * (m + 1)],
                        bdgx_v,
                        start=True, stop=True,
                    )
                    nc.scalar.activation(
                        B_sb[:, 128 * m:128 * m + NS18], XPm[:, 0:NS18],
                        mybir.ActivationFunctionType.Copy,
                    )
                APs = []
                for h in range(2):
                    lo, hi = (0, HL) if h == 0 else (HL, 324)
                    w = hi - lo
                    APh = app.tile([128, 324 - HL], F32, tag="ap",
                                   name=f"AP{tag}{h}")
                    nc.tensor.matmul(
                        APh[:, 0:w], B_sb[:, 0:128], gyzA_sb[:, lo:hi],
                        start=True, stop=False,
                    )
                    nc.tensor.matmul(
                        APh[:, 0:w], B_sb[:, 128:256], gyzA_sb[:, 324 + lo:324 + hi],
                        start=False, stop=False,
                    )
                    nc.tensor.matmul(
                        APh[:, 0:w], B_sb[0:68, 256:384], gyzB_sb[:, lo:hi],
                        start=False, stop=True,
                    )
                    APs.append(APh)
                return APs

            def halves(APs, ptiles, act_dst, is_n):
                """mul -> selector matmul -> activation, emitted stage-wise
                per engine so neither engine's FIFO blocks the other half.
                ptiles are separate PSUM tiles (no false WAR deps)."""
                spans = [(0, HL), (HL, 324)]
                for h in range(2):
                    lo, hi = spans[h]
                    nc.vector.tensor_mul(
                        FS_sb[:, lo:hi], phisAM_sb[0:NS18, lo:hi],
                        APs[h][0:NS18, 0:hi - lo],
                    )
                for h in range(2):
                    lo, hi = spans[h]
                    nc.tensor.matmul(
                        ptiles[h][:, 0:hi - lo], sel_v, FS_sb[:, lo:hi],
                        start=True, stop=True,
                    )
                for h in range(2):
                    lo, hi = spans[h]
                    nc.scalar.activation(
                        act_dst[:, lo:hi], ptiles[h][0:18, 0:hi - lo],
                        mybir.ActivationFunctionType.Abs_reciprocal_sqrt
                        if is_n else mybir.ActivationFunctionType.Copy,
                        bias=(eps_sb[:, 0:1] if is_n else 0.0),
                        scale=1.0,
                    )

            # ---- pass 1 ----
            A1s = filters(phisAM_sb, "p1")
            if USE_ARSQRT:
                RPh = [mp.tile([18, 324 - HL], F32, tag=f"ph{h}",
                               name=f"RP{h}") for h in range(2)]
                halves(A1s, RPh, n_view, True)
            else:  # CoreSim path (no Abs_reciprocal_sqrt there)
                RP = mp.tile([18, 324], F32)
                for h in range(2):
                    lo, hi = (0, HL) if h == 0 else (HL, 324)
                    nc.vector.tensor_mul(
                        FS_sb[:, lo:hi], phisAM_sb[0:NS18, lo:hi],
                        A1s[h][0:NS18, 0:hi - lo],
                    )
                    nc.tensor.matmul(
                        RP[:, lo:hi], sel_v, FS_sb[:, lo:hi],
                        start=(h == 0), stop=True, skip_group_check=(h > 0),
                    )
                nc.scalar.activation(
                    lnr_sb[:], RP[0:18, :],
                    mybir.ActivationFunctionType.Ln,
                    bias=eps_sb[:, 0:1], scale=1.0,
                )
                nc.scalar.activation(
                    n_view, lnr_sb[:],
                    mybir.ActivationFunctionType.Exp, scale=-0.5,
                )

            nc.sync.dma_start(outn[:], n_view)
            for h in range(2):
                lo, hi = (0, HL) if h == 0 else (HL, 324)
                nc.vector.tensor_mul(
                    nh_sb[:, lo:hi], n_view[:, lo:hi], h1cb_sb[:, lo:hi]
                )
            NREP = mp.tile([128, 324], F32)
            nc.tensor.matmul(
                NREP[:, :], selrepb_sb[:, :], nh_sb[:, :],
                start=True, stop=True,
            )

            # ---- pass 2 (W2 multiplied in chunks so the x-matmuls chase) ----
            for m in range(3):
                lo, hi = 128 * m, min(128 * (m + 1), 324)
                nc.vector.tensor_mul(
                    W2_sb[0:NS18, lo:hi], phisAM_sb[0:NS18, lo:hi],
                    NREP[0:NS18, lo:hi],
                )
            A2s = filters(W2_sb, "p2")
            if USE_ARSQRT:
                FPh = [mp.tile([18, 324 - HL], F32, tag=f"ph{h}",
                               name=f"FP{h}") for h in range(2)]
                halves(A2s, FPh, outb_sb, False)
            else:
                FP = mp.tile([18, 324], F32)
                for h in range(2):
                    lo, hi = (0, HL) if h == 0 else (HL, 324)
                    nc.vector.tensor_mul(
                        FS_sb[:, lo:hi], phisAM_sb[0:NS18, lo:hi],
                        A2s[h][0:NS18, 0:hi - lo],
                    )
                    nc.tensor.matmul(
                        FP[:, lo:hi], sel_v, FS_sb[:, lo:hi],
                        start=(h == 0), stop=True, skip_group_check=(h > 0),
                    )
                nc.scalar.activation(
                    outb_sb[:, 0:324], FP[0:18, :],
                    mybir.ActivationFunctionType.Copy,
                )

            # host does loss_part = sum(FP * n * (1-H1c));
            # the two output halves ride different queues in parallel
            nc.sync.dma_start(outf[:, 0:HL], outb_sb[:, 0:HL])
            nc.scalar.dma_start(outf[:, HL:324], outb_sb[:, HL:324])

    nc.compile()
    return nc


def _host_prep(I, U):
    """Per-core input tensors. Returns (in_maps, m1c list)."""
    g = np.arange(XD, dtype=np.float64)
    G1 = np.exp(-0.5 * ((g[:, None] - g[None, :]) / ALPHA) ** 2)
    yzi = np.arange(324)
    yy, zz = yzi // XD, yzi % XD
    GYZ = G1[yy[:, None], yy[None, :]] * G1[zz[:, None], zz[None, :]]  # [324,324]
    gyzA_in = np.zeros((128, 648), BF)
    gyzA_in[:, 0:324] = GYZ[0:128, :].astype(BF)
    gyzA_in[:, 324:648] = GYZ[128:256, :].astype(BF)
    gyzB_in = GYZ[256:324, :].astype(BF)  # [68, 324]

    selrepb_in = np.zeros((18, 128), BF)
    for v in range(P):
        selrepb_in[:, 18 * v:18 * v + 18] = np.eye(XD, dtype=np.float32).astype(BF)

    in_maps = []
    m1cs = []
    for k in range(8):
        b, c = divmod(k, 4)
        w = I[b].reshape(3, N).astype(np.float64) / BETA
        sF = np.exp(-0.5 * (w * w).sum(0))
        Phis = np.stack(
            [np.sqrt(1.0 / (math.factorial(a) * math.factorial(bb) * math.factorial(cc)))
             * (w[0] ** a) * (w[1] ** bb) * (w[2] ** cc) * sF
             for (a, bb, cc) in ALPHAS], 0)  # [P, N]
        Uf = U[b].reshape(C, N).astype(np.float64)
        Uf = Uf - Uf.max(0)
        e = np.exp(Uf)
        H1 = e / e.sum(0)
        hc = H1[c]

        phisAM_in = np.zeros((128, PAW), BF)
        for v in range(P):
            rows = slice(18 * v, 18 * v + 18)
            phisAM_in[rows, 0:324] = Phis[v].reshape(XD, 324).astype(BF)
            phisAM_in[rows, 384 + 18 * v:384 + 18 * v + 18] = G1.astype(BF)
            phisAM_in[rows, 456:474] = np.eye(XD, dtype=np.float32).astype(BF)

        in_maps.append({
            "phisAM": phisAM_in,
            "h1cb": hc.reshape(XD, 324).astype(BF),
            "gyzA": gyzA_in,
            "gyzB": gyzB_in,
            "selrepb": selrepb_in,
        })
        m1cs.append((1.0 - hc).reshape(XD, 324))
    return in_maps, m1cs


def kernel(I, U):
    global LAST_RESULT
    if "nc" not in _compiled:
        _compiled["nc"] = _build()
    nc = _compiled["nc"]

    I = np.asarray(I, np.float32)
    U = np.asarray(U, np.float32)
    in_maps, m1cs = _host_prep(I, U)

    res = bass_utils.run_bass_kernel_spmd(
        nc, in_maps, core_ids=list(range(8)), trace=TRACE
    )
    LAST_RESULT = res

    loss = 0.0
    for k in range(8):
        fp = res.results[k]["outf"].astype(np.float64)
        nn = res.results[k]["outn"].astype(np.float64)
        loss += (fp * nn * m1cs[k]).sum()
    return np.float32(loss)

